# revision 48
# baseline (speedup 1.0000x reference)
"""Trainium2 Bass kernel for nn_Beta_score2 (gnn_message_passing).

Computation (per batch element b):
  nodes   = 6 feature vectors x_k (padded to 2048; padding never contributes)
  temp_k  = tanh(x_k @ W[:, :d_k]^T + b)          # [512]
  score_k = temp_k . h_n                           # scalar
  beta    = softmax(score)                         # [6]
  z       = sum_k beta_k * pad(x_k)                # [2048], cols 1024: always 0

Sharding: data-parallel over batch, B=8192 -> 1024 per core on 8 cores.

Per-core pipeline, three batch chunks (384/384/256 columns):
  stage 1: PE matmuls W^T-chunks x xT-slices -> PSUM [128o, W];
           ACT fused bias+tanh -> temp^T in SBUF (bf16).
  stage 2: score matmuls with slim h-tiles [128, 6] (LDWEIGHTS ~5ns)
           accumulating all 24 (node, oc) pieces into psum rows 0:6;
           emitted as a 4-matmul burst one node late so the PE always has
           the next node's mains queued first.  PE-transpose to batch-major,
           softmax on ACT/DVE -> beta [128b, nj*6].
  stage 3: batch-major z = sum_k beta_k * x_k on DVE as tensor_scalar_mul
           (4x mode) + tensor_tensor add (2x) pairs; the tail chunk splits
           its two groups PE-diag / DVE so the ending is short.
  xt is stored full-width [128, 4, 1024] per k-group: chunks are free SBUF
  column slices, so x loads are 8 big DMAs (+ first-slice splits for an
  early start).  Sync queue carries ONLY what the first matmuls wait on
  (per-queue batched semaphore waits); everything else streams on GpSimd
  in exact consumption order.
  head: PE warm-up junk matmuls run during the initial DMA wait so the HAM
        clock gate releases before real matmuls; a few junk matmuls gated on
        the last score psum keep the PE warm through the tail softmax window.

Host pre-tiles every DRAM tensor so each dma_start is a contiguous copy.
"""

import os
import sys
import types

import numpy as np

B_TOTAL = 8192
NCORES = 8
BLOC = B_TOTAL // NCORES  # 1024
OUT = 512
DW = 1024                 # only W[:, :1024] is ever used
NODES = 6
NODE_OFF = [0, 1024, 1536, 2048, 2560, 3584]
NODE_DIM = [1024, 512, 512, 512, 1024, 512]
NODE_ORDER = (1, 2, 0, 4, 3, 5)   # 1 first (single x-group), then 8-kc nodes
                                  # so the tanh pipeline gets slack
GK = 4                    # xt group: [128, GK, 1024]
NG = 8                    # 32 kc-chunks / GK
CHUNKS = [(0, 512), (512, 768), (768, 1024)]
FIRST_SLICE = 512         # xt tiles stream in [0:512] + [512:1024] pieces

MM_DTYPE = os.environ.get("KERNEL_MM_DTYPE", "bfloat16")
S3_DTYPE = os.environ.get("KERNEL_S3_DTYPE", "float16")
SC_DTYPE = os.environ.get("KERNEL_SC_DTYPE", "bfloat16")  # score-matmul dtype
N_JUNK = int(os.environ.get("KERNEL_N_JUNK", "26"))
JUNK_F = int(os.environ.get("KERNEL_JUNK_F", "256"))
N_JUNK2 = int(os.environ.get("KERNEL_N_JUNK2", "6"))
N_JUNK3 = int(os.environ.get("KERNEL_N_JUNK3", "8"))

LAST_EXEC_TIME_NS = None
LAST_RESULT = None

_cache = {}


def _install_ntff_hook():
    """run_bass_kernel_spmd(trace=True) under axon needs antenv.axon_hooks,
    which this image lacks; synthesize it from trn_agent_boot."""
    if "antenv.axon_hooks" in sys.modules:
        return
    try:
        import antenv
        import trn_agent_boot.trn_boot as tb
    except Exception:
        return
    mod = types.ModuleType("antenv.axon_hooks")
    _hook = tb._ntff_profile_via_ctypes("/opt/axon/libaxon_pjrt.so")
    mod.get_axon_ntff_profile_hook = lambda: _hook
    mod.set_axon_ntff_profile_hook = lambda h: None
    sys.modules["antenv.axon_hooks"] = mod
    antenv.axon_hooks = mod


def _build(mm_dtype_name, s3_dtype_name, sc_dtype_name):
    from contextlib import ExitStack

    import concourse.bacc as bacc
    import concourse.mybir as mybir
    import concourse.tile as tile

    f32 = mybir.dt.float32
    mm_dt = getattr(mybir.dt, mm_dtype_name)
    s3_dt = getattr(mybir.dt, s3_dtype_name)
    sc_dt = getattr(mybir.dt, sc_dtype_name)
    # fp8 tanh outputs let the score matmuls run DoubleRow (two oc-chunks
    # of the o-contraction per pass)
    sc_dr = sc_dtype_name.startswith("float8")
    DR = mybir.MatmulPerfMode.DoubleRow if sc_dr else None

    nc = bacc.Bacc("TRN2", target_bir_lowering=False, debug=False)
    xt_d = nc.dram_tensor("xt", [NG, 128, GK, BLOC], mm_dt, kind="ExternalInput").ap()
    xb_d = nc.dram_tensor("xb", [8, 128, 4096], s3_dt, kind="ExternalInput").ap()
    wt_d = nc.dram_tensor("wt", [2, 128, 4 * OUT], mm_dt, kind="ExternalInput").ap()
    bias_d = nc.dram_tensor("bias", [128, 4], f32, kind="ExternalInput").ap()
    h6_d = nc.dram_tensor("h6", [128, 24, 16], sc_dt, kind="ExternalInput").ap()
    eye_d = nc.dram_tensor("eye", [6, 6], f32, kind="ExternalInput").ap()
    eye128_d = nc.dram_tensor("eye128", [128, 128], s3_dt, kind="ExternalInput").ap()
    z_d = nc.dram_tensor("z", [8, 128, DW], s3_dt, kind="ExternalOutput").ap()

    Tanh = mybir.ActivationFunctionType.Tanh
    Exp = mybir.ActivationFunctionType.Exp
    Copy = mybir.ActivationFunctionType.Copy
    Add = mybir.AluOpType.add

    with tile.TileContext(nc) as tc, ExitStack() as ctx:
        const = ctx.enter_context(tc.tile_pool(name="const", bufs=1))
        wt_lo = const.tile([128, 4, OUT], mm_dt)
        wt_hi = const.tile([128, 4, OUT], mm_dt)
        bias_t = const.tile([128, 4], f32)
        h6_t = const.tile([128, 24, 16], sc_dt)
        eye_t = const.tile([6, 6], f32)
        eye128_t = const.tile([128, 128], s3_dt)
        junk_w = const.tile([128, 128], mm_dt, name="junk_w")
        junk_x = const.tile([128, JUNK_F], mm_dt, name="junk_x")

        pre_ps = ctx.enter_context(tc.tile_pool(name="pre", bufs=5, space="PSUM"))
        score_ps = ctx.enter_context(tc.tile_pool(name="score", bufs=2, space="PSUM"))
        tp_ps = ctx.enter_context(tc.tile_pool(name="tp", bufs=1, space="PSUM"))
        temps = ctx.enter_context(tc.tile_pool(name="temps", bufs=8))
        small = ctx.enter_context(tc.tile_pool(name="small", bufs=2))
        zpool = ctx.enter_context(tc.tile_pool(name="zpool", bufs=3))
        stmp = ctx.enter_context(tc.tile_pool(name="stmp", bufs=4))
        diag_pool = ctx.enter_context(tc.tile_pool(name="diag", bufs=8))

        # ---- PE warm-up: junk matmuls release the HAM clock gate while the
        # first DMAs are in flight.  memsets on GpSimd (idle at start).
        nc.gpsimd.memset(junk_w[:], 0.0)
        nc.gpsimd.memset(junk_x[:], 0.0)
        jp_a = pre_ps.tile([128, 512], f32, name="jp_a", tag="ps")
        jp = score_ps.tile([128, 512], f32, name="junk_ps", tag="sc")
        for i in range(4):
            nc.tensor.matmul(jp_a[:, :JUNK_F], junk_w[:], junk_x[:], start=True, stop=True)
        for i in range(N_JUNK - 4):
            nc.tensor.matmul(jp[:, :JUNK_F], junk_w[:], junk_x[:], start=True, stop=True)
        # dummy tanh pulls the ACT table load into the DMA-wait window
        warm_t = temps.tile([128, JUNK_F], mm_dt, tag="warm", name="warm_t")
        nc.scalar.activation(warm_t[:], junk_x[:], Tanh, bias=0.0, scale=1.0)

        # ---- DMA dispatch.
        xts_tiles = {}

        def xt_tile(g):
            if g not in xts_tiles:
                xts_tiles[g] = const.tile([128, GK, BLOC], mm_dt, name=f"xt_{g}")
            return xts_tiles[g]

        def load_xt_a(g, eng):
            eng.dma_start(xt_tile(g)[:, :, 0:FIRST_SLICE], xt_d[g][:, :, 0:FIRST_SLICE])

        def load_xt_b(g, eng):
            eng.dma_start(
                xt_tile(g)[:, :, FIRST_SLICE:BLOC], xt_d[g][:, :, FIRST_SLICE:BLOC]
            )

        xb_tiles = {}

        def load_xb(j, eng):
            t = const.tile([128, 4096], s3_dt, name=f"xb_{j}")
            eng.dma_start(t[:], xb_d[j])
            xb_tiles[j] = t

        # Sync carries ONLY what the first matmul chain waits on (the per-
        # queue batched semaphore wait makes the first consumer wait for
        # everything issued ahead of it on that queue).
        nc.sync.dma_start(wt_lo[:, 0:2, :], wt_d[0][:, 0 : 2 * OUT])
        nc.sync.dma_start(
            xt_tile(2)[:, 0:2, 0:FIRST_SLICE], xt_d[2][:, 0:2, 0:FIRST_SLICE]
        )
        gate1 = const.tile([1, 1], f32, name="gate1")
        gate2 = const.tile([1, 1], f32, name="gate2")
        nc.vector.tensor_copy(gate1[:], jp_a[0:1, 0:1])
        nc.gpsimd.tensor_copy(gate2[:], gate1[:])
        nc.gpsimd.dma_start(
            xt_tile(2)[:, 2:4, 0:FIRST_SLICE], xt_d[2][:, 2:4, 0:FIRST_SLICE]
        )
        nc.gpsimd.dma_start(bias_t[:], bias_d[:, :])
        nc.gpsimd.dma_start(wt_lo[:, 2:4, :], wt_d[0][:, 2 * OUT : 4 * OUT])
        load_xt_a(3, nc.gpsimd)
        load_xt_a(0, nc.gpsimd)
        nc.gpsimd.dma_start(wt_hi[:], wt_d[1])
        load_xt_a(1, nc.gpsimd)
        nc.gpsimd.dma_start(h6_t[:], h6_d[:, :, :])
        load_xt_a(5, nc.gpsimd)
        load_xt_a(6, nc.gpsimd)
        nc.gpsimd.dma_start(eye_t[:], eye_d[:, :])
        load_xt_a(4, nc.gpsimd)
        load_xt_a(7, nc.gpsimd)
        # remainders + xb interleaved in consumption order
        load_xt_b(2, nc.gpsimd)
        load_xt_b(3, nc.gpsimd)
        load_xb(0, nc.gpsimd)
        load_xt_b(0, nc.gpsimd)
        load_xt_b(1, nc.gpsimd)
        load_xb(1, nc.gpsimd)
        load_xt_b(5, nc.gpsimd)
        load_xt_b(6, nc.gpsimd)
        load_xb(2, nc.gpsimd)
        load_xt_b(4, nc.gpsimd)
        load_xb(3, nc.gpsimd)
        load_xt_b(7, nc.gpsimd)
        nc.gpsimd.dma_start(eye128_t[:], eye128_d[:, :])
        load_xb(4, nc.gpsimd)
        load_xb(5, nc.gpsimd)
        load_xb(6, nc.gpsimd)
        load_xb(7, nc.gpsimd)

        def xts(kc, c0, c1):
            return xts_tiles[kc // GK][:, kc % GK, c0:c1]

        def wts(kc, oc):
            w = wt_lo if kc < 4 else wt_hi
            return w[:, kc % 4, oc * 128 : (oc + 1) * 128]

        # Score matmuls for a node are emitted as a 4-matmul burst one node
        # late, so the PE always has the next node's mains queued first.
        pending_sc = []

        def flush_sc():
            while pending_sc:
                sc, n, tts, st_n, sp_n = pending_sc.pop(0)
                if sc_dr:
                    for qi in range(2):
                        nc.tensor.matmul(
                            sc[0:6, :],
                            h6_t[:, n * 4 + 2 * qi : n * 4 + 2 * qi + 2, 0:6],
                            tts[qi][:],
                            start=(st_n and qi == 0),
                            stop=(sp_n and qi == 1),
                            perf_mode=DR,
                        )
                else:
                    for oc in range(4):
                        nc.tensor.matmul(
                            sc[0:6, :],
                            h6_t[:, n * 4 + oc, 0:6],
                            tts[oc][:],
                            start=(st_n and oc == 0),
                            stop=(sp_n and oc == 3),
                        )

        scs = {}

        def emit_stage1_node(ci, n):
            c0, c1 = CHUNKS[ci]
            w = c1 - c0
            if n == NODE_ORDER[0]:
                scs[ci] = score_ps.tile([128, w], f32, name=f"sc_{ci}", tag="sc")
            sc = scs[ci]
            nk = NODE_DIM[n] // 128
            off = NODE_OFF[n] // 128
            tts = []
            pair = None
            for oc in range(4):
                ps = pre_ps.tile([128, w], f32, tag="ps")
                for kc in range(nk):
                    nc.tensor.matmul(
                        ps[:],
                        wts(kc, oc),
                        xts(off + kc, c0, c1),
                        start=(kc == 0),
                        stop=(kc == nk - 1),
                    )
                if oc == 1:
                    flush_sc()
                if sc_dr:
                    if oc % 2 == 0:
                        pair = temps.tile([128, 2, w], sc_dt, tag="tt", name="tt")
                        tts.append(pair)
                    tt_dst = pair[:, oc % 2, :]
                else:
                    tt = temps.tile([128, w], sc_dt, tag="tt", name="tt")
                    tts.append(tt)
                    tt_dst = tt[:]
                nc.scalar.activation(
                    tt_dst, ps[:], Tanh, bias=bias_t[:, oc : oc + 1], scale=1.0
                )
            pending_sc.append(
                (sc, n, tts, n == NODE_ORDER[0], n == NODE_ORDER[-1])
            )

        def dve_group(j, beta, jrel, split_dma=False):
            # z = sum_k beta_k * x_k on DVE as ts_mul(4x) + tt_add(2x) pairs
            xb = xb_tiles[j]
            bf = beta[:, jrel * 6 : jrel * 6 + 6]
            z = zpool.tile([128, DW], s3_dt, tag="z", name=f"z_{j}")
            nc.vector.tensor_scalar_mul(z[:], xb[:, 0:DW], bf[:, 0:1])
            t4 = stmp.tile([128, DW], s3_dt, tag="tmpa")
            nc.vector.tensor_scalar_mul(t4[:], xb[:, 2560:3584], bf[:, 4:5])
            nc.vector.tensor_tensor(z[:], z[:], t4[:], Add)
            if split_dma:
                # z[512:] is final after the first add; stream it out early so
                # the kernel-final transfer is only the 128KB z[0:512] half
                nc.sync.dma_start(z_d[j][:, 512:DW], z[:, 512:DW])
            for k, lo in ((1, 1024), (2, 1536), (3, 2048), (5, 3584)):
                tk = stmp.tile([128, 512], s3_dt, tag="tmpb")
                nc.vector.tensor_scalar_mul(tk[:], xb[:, lo : lo + 512], bf[:, k : k + 1])
                nc.vector.tensor_tensor(z[:, 0:512], z[:, 0:512], tk[:], Add)
            if split_dma:
                nc.sync.dma_start(z_d[j][:, 0:512], z[:, 0:512])
            else:
                nc.sync.dma_start(z_d[j], z[:])

        def pe_group(j, beta, jrel):
            # diag-matmul route: diag(beta_k) @ xb slices on the PE.
            # diags built on ACT (DVE is the backlogged engine at the tail);
            # z streamed out in halves so the final transfer is small.
            xb = xb_tiles[j]
            bf = beta[:, jrel * 6 : jrel * 6 + 6]
            diags = []
            for k in range(6):
                d = diag_pool.tile([128, 128], s3_dt, tag="dg", name=f"dg{j}_{k}")
                nc.scalar.activation(
                    d[:], eye128_t[:], Copy, bias=0.0, scale=bf[:, k : k + 1]
                )
                diags.append(d)
            z = zpool.tile([128, DW], s3_dt, tag="z", name=f"zd_{j}")
            za = pre_ps.tile([128, 512], f32, tag="ps", name=f"za_{j}")
            for i, (k, lo) in enumerate(
                ((0, 0), (1, 1024), (2, 1536), (3, 2048), (4, 2560), (5, 3584))
            ):
                nc.tensor.matmul(
                    za[:], diags[k][:], xb[:, lo : lo + 512],
                    start=(i == 0), stop=(i == 5),
                )
            nc.scalar.copy(z[:, 0:512], za[:])
            nc.sync.dma_start(z_d[j][:, 0:512], z[:, 0:512])
            zb = pre_ps.tile([128, 512], f32, tag="ps", name=f"zb_{j}")
            nc.tensor.matmul(zb[:], diags[0][:], xb[:, 512:1024], start=True, stop=False)
            nc.tensor.matmul(zb[:], diags[4][:], xb[:, 3072:3584], start=False, stop=True)
            nc.scalar.copy(z[:, 512:1024], zb[:])
            nc.sync.dma_start(z_d[j][:, 512:DW], z[:, 512:DW])

        def emit_stage23(ci):
            c0, c1 = CHUNKS[ci]
            w = c1 - c0
            nj = w // 128
            jbase = c0 // 128
            tail = ci == len(CHUNKS) - 1
            sc = scs[ci]
            # ---------- softmax over the 6 nodes (batch-major) ----------
            if tail:
                # junk matmuls gated on the final scores keep the PE warm
                # through the softmax window for the tail diag-matmuls
                nc.scalar.copy(junk_x[0:1, 0:1], sc[0:1, 0:1])
                jp2 = score_ps.tile([128, 512], f32, tag="sc", name="jp2")
                for i in range(N_JUNK2):
                    nc.tensor.matmul(
                        jp2[:, :JUNK_F], junk_w[:], junk_x[:], start=True, stop=True
                    )
            sc_sb = small.tile([6, w], f32, tag="sc_sb")
            nc.scalar.copy(sc_sb[:], sc[0:6, :])
            tp = tp_ps.tile([128, nj * 6], f32)
            for j in range(nj):
                nc.tensor.transpose(
                    tp[:, j * 6 : (j + 1) * 6],
                    sc_sb[:, j * 128 : (j + 1) * 128],
                    eye_t[:],
                )
            expt = small.tile([128, nj * 6], f32, tag="expt")
            sumexp = small.tile([128, nj], f32, tag="sumexp")
            nc.scalar.activation(expt[:], tp[:], Exp)
            if tail:
                # second junk batch bridges exp -> diag-build
                nc.scalar.copy(junk_x[0:1, 0:1], expt[0:1, 0:1])
                jp3 = score_ps.tile([128, 512], f32, tag="sc", name="jp3")
                for i in range(N_JUNK3):
                    nc.tensor.matmul(
                        jp3[:, :JUNK_F], junk_w[:], junk_x[:], start=True, stop=True
                    )
            nc.vector.tensor_reduce(
                sumexp[:],
                expt[:].rearrange("p (j k) -> p j k", j=nj),
                axis=mybir.AxisListType.X,
                op=mybir.AluOpType.add,
            )
            rec = small.tile([128, nj], f32, tag="rec")
            nc.vector.reciprocal(rec[:], sumexp[:])
            beta = small.tile([128, nj * 6], f32, tag="beta")
            for j in range(nj):
                nc.vector.tensor_scalar_mul(
                    beta[:, j * 6 : (j + 1) * 6],
                    expt[:, j * 6 : (j + 1) * 6],
                    rec[:, j : j + 1],
                )
            # ---------- stage 3 ----------
            if not tail:
                for jrel in range(nj):
                    dve_group(jbase + jrel, beta, jrel)
            else:
                pe_group(jbase + 1, beta, 1)
                dve_group(jbase, beta, 0, split_dma=True)

        for n in NODE_ORDER:
            emit_stage1_node(0, n)
        for ci in range(1, len(CHUNKS)):
            emit_stage1_node(ci, NODE_ORDER[0])
            emit_stage23(ci - 1)
            for n in NODE_ORDER[1:]:
                emit_stage1_node(ci, n)
        flush_sc()
        emit_stage23(len(CHUNKS) - 1)

    nc.compile()
    return nc


def _get_nc():
    key = (MM_DTYPE, S3_DTYPE, SC_DTYPE)
    if key not in _cache:
        _cache[key] = _build(*key)
    return _cache[key]


def kernel(result_ls, result_A, result_lm, result_AT, result_ds, result_dm, W, b, h_n):
    global LAST_EXEC_TIME_NS, LAST_RESULT
    _install_ntff_hook()
    from concourse.bass_utils import run_bass_kernel_spmd

    import concourse.mybir as mybir

    nc = _get_nc()
    mm_np = mybir.dt.np(getattr(mybir.dt, MM_DTYPE))
    s3_np = mybir.dt.np(getattr(mybir.dt, S3_DTYPE))
    sc_np = mybir.dt.np(getattr(mybir.dt, SC_DTYPE))

    x = np.concatenate(
        [
            np.asarray(t, dtype=np.float32).reshape(B_TOTAL, -1)
            for t in (result_ls, result_A, result_lm, result_AT, result_ds, result_dm)
        ],
        axis=1,
    )  # [8192, 4096]
    W = np.asarray(W, dtype=np.float32)
    b = np.asarray(b, dtype=np.float32)
    h_n = np.asarray(h_n, dtype=np.float32)

    wT = np.ascontiguousarray(W[:, :DW].T).astype(mm_np)       # [1024, 512]
    wt = wT.reshape(2, 4, 128, OUT).transpose(0, 2, 1, 3)      # [2, 128, 4, 512]
    wt = np.ascontiguousarray(wt)
    bias = np.ascontiguousarray(b.reshape(4, 128).T)           # [128, 4]
    # h6[p, n*4 + oc, m] = h[oc*128 + p] if m == n else 0
    h6 = np.zeros((128, 24, 16), dtype=np.float32)
    for n in range(NODES):
        for oc in range(4):
            h6[:, n * 4 + oc, n] = h_n[oc * 128 : (oc + 1) * 128, 0]
    h6 = h6.astype(sc_np)
    eye = np.eye(6, dtype=np.float32)
    eye128 = np.eye(128, dtype=s3_np)

    in_maps = []
    for c in range(NCORES):
        xc = x[c * BLOC : (c + 1) * BLOC]                      # [1024, 4096]
        xT = np.ascontiguousarray(xc.T)                        # [4096, 1024]
        # xt[g, p, jkc, b] = xT[g*512 + jkc*128 + p, b]
        xt = np.ascontiguousarray(
            xT.reshape(NG, GK, 128, BLOC).transpose(0, 2, 1, 3)
        ).astype(mm_np)
        in_maps.append(
            {
                "xt": xt,
                "xb": np.ascontiguousarray(xc.reshape(8, 128, 4096)).astype(s3_np),
                "wt": wt.reshape(2, 128, 4 * OUT),
                "bias": bias,
                "h6": h6,
                "eye": eye,
                "eye128": eye128,
            }
        )

    res = run_bass_kernel_spmd(nc, in_maps, list(range(NCORES)))
    LAST_RESULT = res
    LAST_EXEC_TIME_NS = res.exec_time_ns

    out = np.zeros((B_TOTAL, 1, 2048), dtype=np.float32)
    for c in range(NCORES):
        zc = res.results[c]["z"]                               # [8, 128, 1024]
        out[c * BLOC : (c + 1) * BLOC, 0, :DW] = zc.reshape(BLOC, DW).astype(np.float32)
    return out


# revision 49
# speedup vs baseline: 1.0043x; 1.0043x over previous
"""Trainium2 Bass kernel for nn_Beta_score2 (gnn_message_passing).

Computation (per batch element b):
  nodes   = 6 feature vectors x_k (padded to 2048; padding never contributes)
  temp_k  = tanh(x_k @ W[:, :d_k]^T + b)          # [512]
  score_k = temp_k . h_n                           # scalar
  beta    = softmax(score)                         # [6]
  z       = sum_k beta_k * pad(x_k)                # [2048], cols 1024: always 0

Sharding: data-parallel over batch, B=8192 -> 1024 per core on 8 cores.

Per-core pipeline, three batch chunks (384/384/256 columns):
  stage 1: PE matmuls W^T-chunks x xT-slices -> PSUM [128o, W];
           ACT fused bias+tanh -> temp^T in SBUF (bf16).
  stage 2: score matmuls with slim h-tiles [128, 6] (LDWEIGHTS ~5ns)
           accumulating all 24 (node, oc) pieces into psum rows 0:6;
           emitted as a 4-matmul burst one node late so the PE always has
           the next node's mains queued first.  PE-transpose to batch-major,
           softmax on ACT/DVE -> beta [128b, nj*6].
  stage 3: batch-major z = sum_k beta_k * x_k on DVE as tensor_scalar_mul
           (4x mode) + tensor_tensor add (2x) pairs; the tail chunk splits
           its two groups PE-diag / DVE so the ending is short.
  xt is stored full-width [128, 4, 1024] per k-group: chunks are free SBUF
  column slices, so x loads are 8 big DMAs (+ first-slice splits for an
  early start).  Sync queue carries ONLY what the first matmuls wait on
  (per-queue batched semaphore waits); everything else streams on GpSimd
  in exact consumption order.
  head: PE warm-up junk matmuls run during the initial DMA wait so the HAM
        clock gate releases before real matmuls; a few junk matmuls gated on
        the last score psum keep the PE warm through the tail softmax window.

Host pre-tiles every DRAM tensor so each dma_start is a contiguous copy.
"""

import os
import sys
import types

import numpy as np

B_TOTAL = 8192
NCORES = 8
BLOC = B_TOTAL // NCORES  # 1024
OUT = 512
DW = 1024                 # only W[:, :1024] is ever used
NODES = 6
NODE_OFF = [0, 1024, 1536, 2048, 2560, 3584]
NODE_DIM = [1024, 512, 512, 512, 1024, 512]
NODE_ORDER = (1, 2, 0, 4, 3, 5)   # 1 first (single x-group), then 8-kc nodes
                                  # so the tanh pipeline gets slack
GK = 4                    # xt group: [128, GK, 1024]
NG = 8                    # 32 kc-chunks / GK
CHUNKS = [(0, 512), (512, 768), (768, 1024)]
FIRST_SLICE = 512         # xt tiles stream in [0:512] + [512:1024] pieces

MM_DTYPE = os.environ.get("KERNEL_MM_DTYPE", "bfloat16")
S3_DTYPE = os.environ.get("KERNEL_S3_DTYPE", "float16")
SC_DTYPE = os.environ.get("KERNEL_SC_DTYPE", "bfloat16")  # score-matmul dtype
N_JUNK = int(os.environ.get("KERNEL_N_JUNK", "26"))
JUNK_F = int(os.environ.get("KERNEL_JUNK_F", "256"))
N_JUNK2 = int(os.environ.get("KERNEL_N_JUNK2", "6"))
N_JUNK3 = int(os.environ.get("KERNEL_N_JUNK3", "8"))

LAST_EXEC_TIME_NS = None
LAST_RESULT = None

_cache = {}


def _install_ntff_hook():
    """run_bass_kernel_spmd(trace=True) under axon needs antenv.axon_hooks,
    which this image lacks; synthesize it from trn_agent_boot."""
    if "antenv.axon_hooks" in sys.modules:
        return
    try:
        import antenv
        import trn_agent_boot.trn_boot as tb
    except Exception:
        return
    mod = types.ModuleType("antenv.axon_hooks")
    _hook = tb._ntff_profile_via_ctypes("/opt/axon/libaxon_pjrt.so")
    mod.get_axon_ntff_profile_hook = lambda: _hook
    mod.set_axon_ntff_profile_hook = lambda h: None
    sys.modules["antenv.axon_hooks"] = mod
    antenv.axon_hooks = mod


def _build(mm_dtype_name, s3_dtype_name, sc_dtype_name):
    from contextlib import ExitStack

    import concourse.bacc as bacc
    import concourse.mybir as mybir
    import concourse.tile as tile

    f32 = mybir.dt.float32
    mm_dt = getattr(mybir.dt, mm_dtype_name)
    s3_dt = getattr(mybir.dt, s3_dtype_name)
    sc_dt = getattr(mybir.dt, sc_dtype_name)
    # fp8 tanh outputs let the score matmuls run DoubleRow (two oc-chunks
    # of the o-contraction per pass)
    sc_dr = sc_dtype_name.startswith("float8")
    DR = mybir.MatmulPerfMode.DoubleRow if sc_dr else None

    nc = bacc.Bacc("TRN2", target_bir_lowering=False, debug=False)
    xt_d = nc.dram_tensor("xt", [NG, 128, GK, BLOC], mm_dt, kind="ExternalInput").ap()
    xb_d = nc.dram_tensor("xb", [8, 128, 4096], s3_dt, kind="ExternalInput").ap()
    wt_d = nc.dram_tensor("wt", [2, 128, 4 * OUT], mm_dt, kind="ExternalInput").ap()
    bias_d = nc.dram_tensor("bias", [128, 4], f32, kind="ExternalInput").ap()
    h6_d = nc.dram_tensor("h6", [128, 24, 16], sc_dt, kind="ExternalInput").ap()
    eye_d = nc.dram_tensor("eye", [6, 6], f32, kind="ExternalInput").ap()
    eye128_d = nc.dram_tensor("eye128", [128, 128], s3_dt, kind="ExternalInput").ap()
    z_d = nc.dram_tensor("z", [8, 128, DW], s3_dt, kind="ExternalOutput").ap()

    Tanh = mybir.ActivationFunctionType.Tanh
    Exp = mybir.ActivationFunctionType.Exp
    Copy = mybir.ActivationFunctionType.Copy
    Add = mybir.AluOpType.add

    with tile.TileContext(nc) as tc, ExitStack() as ctx:
        const = ctx.enter_context(tc.tile_pool(name="const", bufs=1))
        wt_lo = const.tile([128, 4, OUT], mm_dt)
        wt_hi = const.tile([128, 4, OUT], mm_dt)
        bias_t = const.tile([128, 4], f32)
        h6_t = const.tile([128, 24, 16], sc_dt)
        eye_t = const.tile([6, 6], f32)
        eye128_t = const.tile([128, 128], s3_dt)
        junk_w = const.tile([128, 128], mm_dt, name="junk_w")
        junk_x = const.tile([128, JUNK_F], mm_dt, name="junk_x")

        pre_ps = ctx.enter_context(tc.tile_pool(name="pre", bufs=4, space="PSUM"))
        score_ps = ctx.enter_context(tc.tile_pool(name="score", bufs=2, space="PSUM"))
        tp_ps = ctx.enter_context(tc.tile_pool(name="tp", bufs=1, space="PSUM"))
        temps = ctx.enter_context(tc.tile_pool(name="temps", bufs=8))
        small = ctx.enter_context(tc.tile_pool(name="small", bufs=2))
        zpool = ctx.enter_context(tc.tile_pool(name="zpool", bufs=3))
        stmp = ctx.enter_context(tc.tile_pool(name="stmp", bufs=4))
        diag_pool = ctx.enter_context(tc.tile_pool(name="diag", bufs=8))

        # ---- PE warm-up: junk matmuls release the HAM clock gate while the
        # first DMAs are in flight.  memsets on GpSimd (idle at start).
        nc.gpsimd.memset(junk_w[:], 0.0)
        nc.gpsimd.memset(junk_x[:], 0.0)
        jp_a = pre_ps.tile([128, 512], f32, name="jp_a", tag="ps")
        jp = score_ps.tile([128, 512], f32, name="junk_ps", tag="sc")
        for i in range(4):
            nc.tensor.matmul(jp_a[:, :JUNK_F], junk_w[:], junk_x[:], start=True, stop=True)
        for i in range(N_JUNK - 4):
            nc.tensor.matmul(jp[:, :JUNK_F], junk_w[:], junk_x[:], start=True, stop=True)
        # dummy tanh pulls the ACT table load into the DMA-wait window
        warm_t = temps.tile([128, JUNK_F], mm_dt, tag="warm", name="warm_t")
        nc.scalar.activation(warm_t[:], junk_x[:], Tanh, bias=0.0, scale=1.0)

        # ---- DMA dispatch.
        xts_tiles = {}

        def xt_tile(g):
            if g not in xts_tiles:
                xts_tiles[g] = const.tile([128, GK, BLOC], mm_dt, name=f"xt_{g}")
            return xts_tiles[g]

        def load_xt_a(g, eng):
            eng.dma_start(xt_tile(g)[:, :, 0:FIRST_SLICE], xt_d[g][:, :, 0:FIRST_SLICE])

        def load_xt_b(g, eng):
            eng.dma_start(
                xt_tile(g)[:, :, FIRST_SLICE:BLOC], xt_d[g][:, :, FIRST_SLICE:BLOC]
            )

        xb_tiles = {}

        def load_xb(j, eng):
            t = const.tile([128, 4096], s3_dt, name=f"xb_{j}")
            eng.dma_start(t[:], xb_d[j])
            xb_tiles[j] = t

        # Sync carries ONLY what the first matmul chain waits on (the per-
        # queue batched semaphore wait makes the first consumer wait for
        # everything issued ahead of it on that queue).
        nc.sync.dma_start(wt_lo[:, 0:2, :], wt_d[0][:, 0 : 2 * OUT])
        nc.sync.dma_start(
            xt_tile(2)[:, 0:2, 0:FIRST_SLICE], xt_d[2][:, 0:2, 0:FIRST_SLICE]
        )
        gate1 = const.tile([1, 1], f32, name="gate1")
        gate2 = const.tile([1, 1], f32, name="gate2")
        nc.vector.tensor_copy(gate1[:], jp_a[0:1, 0:1])
        nc.gpsimd.tensor_copy(gate2[:], gate1[:])
        nc.gpsimd.dma_start(
            xt_tile(2)[:, 2:4, 0:FIRST_SLICE], xt_d[2][:, 2:4, 0:FIRST_SLICE]
        )
        nc.gpsimd.dma_start(bias_t[:], bias_d[:, :])
        nc.gpsimd.dma_start(wt_lo[:, 2:4, :], wt_d[0][:, 2 * OUT : 4 * OUT])
        load_xt_a(3, nc.gpsimd)
        load_xt_a(0, nc.gpsimd)
        nc.gpsimd.dma_start(wt_hi[:], wt_d[1])
        load_xt_a(1, nc.gpsimd)
        nc.gpsimd.dma_start(h6_t[:], h6_d[:, :, :])
        load_xt_a(5, nc.gpsimd)
        load_xt_a(6, nc.gpsimd)
        nc.gpsimd.dma_start(eye_t[:], eye_d[:, :])
        load_xt_a(4, nc.gpsimd)
        load_xt_a(7, nc.gpsimd)
        # remainders + xb interleaved in consumption order
        load_xt_b(2, nc.gpsimd)
        load_xt_b(3, nc.gpsimd)
        load_xb(0, nc.gpsimd)
        load_xt_b(0, nc.gpsimd)
        load_xt_b(1, nc.gpsimd)
        load_xb(1, nc.gpsimd)
        load_xt_b(5, nc.gpsimd)
        load_xt_b(6, nc.gpsimd)
        load_xb(2, nc.gpsimd)
        load_xt_b(4, nc.gpsimd)
        load_xb(3, nc.gpsimd)
        load_xt_b(7, nc.gpsimd)
        nc.gpsimd.dma_start(eye128_t[:], eye128_d[:, :])
        load_xb(4, nc.gpsimd)
        load_xb(5, nc.gpsimd)
        load_xb(6, nc.gpsimd)
        load_xb(7, nc.gpsimd)

        def xts(kc, c0, c1):
            return xts_tiles[kc // GK][:, kc % GK, c0:c1]

        def wts(kc, oc):
            w = wt_lo if kc < 4 else wt_hi
            return w[:, kc % 4, oc * 128 : (oc + 1) * 128]

        # Score matmuls for a node are emitted as a 4-matmul burst one node
        # late, so the PE always has the next node's mains queued first.
        pending_sc = []

        def flush_sc():
            while pending_sc:
                sc, n, tts, st_n, sp_n = pending_sc.pop(0)
                if sc_dr:
                    for qi in range(2):
                        nc.tensor.matmul(
                            sc[0:6, :],
                            h6_t[:, n * 4 + 2 * qi : n * 4 + 2 * qi + 2, 0:6],
                            tts[qi][:],
                            start=(st_n and qi == 0),
                            stop=(sp_n and qi == 1),
                            perf_mode=DR,
                        )
                else:
                    for oc in range(4):
                        nc.tensor.matmul(
                            sc[0:6, :],
                            h6_t[:, n * 4 + oc, 0:6],
                            tts[oc][:],
                            start=(st_n and oc == 0),
                            stop=(sp_n and oc == 3),
                        )

        scs = {}

        def emit_stage1_node(ci, n):
            c0, c1 = CHUNKS[ci]
            w = c1 - c0
            if n == NODE_ORDER[0]:
                scs[ci] = score_ps.tile([128, w], f32, name=f"sc_{ci}", tag="sc")
            sc = scs[ci]
            nk = NODE_DIM[n] // 128
            off = NODE_OFF[n] // 128
            tts = []
            pair = None
            for oc in range(4):
                ps = pre_ps.tile([128, w], f32, tag="ps")
                for kc in range(nk):
                    nc.tensor.matmul(
                        ps[:],
                        wts(kc, oc),
                        xts(off + kc, c0, c1),
                        start=(kc == 0),
                        stop=(kc == nk - 1),
                    )
                if oc == 1:
                    flush_sc()
                if sc_dr:
                    if oc % 2 == 0:
                        pair = temps.tile([128, 2, w], sc_dt, tag="tt", name="tt")
                        tts.append(pair)
                    tt_dst = pair[:, oc % 2, :]
                else:
                    tt = temps.tile([128, w], sc_dt, tag="tt", name="tt")
                    tts.append(tt)
                    tt_dst = tt[:]
                nc.scalar.activation(
                    tt_dst, ps[:], Tanh, bias=bias_t[:, oc : oc + 1], scale=1.0
                )
            pending_sc.append(
                (sc, n, tts, n == NODE_ORDER[0], n == NODE_ORDER[-1])
            )

        def dve_group(j, beta, jrel, split_dma=False):
            # z = sum_k beta_k * x_k on DVE as ts_mul(4x) + tt_add(2x) pairs
            xb = xb_tiles[j]
            bf = beta[:, jrel * 6 : jrel * 6 + 6]
            z = zpool.tile([128, DW], s3_dt, tag="z", name=f"z_{j}")
            nc.vector.tensor_scalar_mul(z[:], xb[:, 0:DW], bf[:, 0:1])
            t4 = stmp.tile([128, DW], s3_dt, tag="tmpa")
            nc.vector.tensor_scalar_mul(t4[:], xb[:, 2560:3584], bf[:, 4:5])
            nc.vector.tensor_tensor(z[:], z[:], t4[:], Add)
            if split_dma:
                # z[512:] is final after the first add; stream it out early so
                # the kernel-final transfer is only the 128KB z[0:512] half
                nc.sync.dma_start(z_d[j][:, 512:DW], z[:, 512:DW])
            for k, lo in ((1, 1024), (2, 1536), (3, 2048), (5, 3584)):
                tk = stmp.tile([128, 512], s3_dt, tag="tmpb")
                nc.vector.tensor_scalar_mul(tk[:], xb[:, lo : lo + 512], bf[:, k : k + 1])
                nc.vector.tensor_tensor(z[:, 0:512], z[:, 0:512], tk[:], Add)
            if split_dma:
                nc.sync.dma_start(z_d[j][:, 0:512], z[:, 0:512])
            else:
                nc.sync.dma_start(z_d[j], z[:])

        def pe_group(j, beta, jrel):
            # diag-matmul route: diag(beta_k) @ xb slices on the PE.
            # diags built on ACT (DVE is the backlogged engine at the tail);
            # z streamed out in halves so the final transfer is small.
            xb = xb_tiles[j]
            bf = beta[:, jrel * 6 : jrel * 6 + 6]
            diags = []
            for k in range(6):
                d = diag_pool.tile([128, 128], s3_dt, tag="dg", name=f"dg{j}_{k}")
                nc.scalar.activation(
                    d[:], eye128_t[:], Copy, bias=0.0, scale=bf[:, k : k + 1]
                )
                diags.append(d)
            z = zpool.tile([128, DW], s3_dt, tag="z", name=f"zd_{j}")
            za = pre_ps.tile([128, 512], f32, tag="ps", name=f"za_{j}")
            for i, (k, lo) in enumerate(
                ((0, 0), (1, 1024), (2, 1536), (3, 2048), (4, 2560), (5, 3584))
            ):
                nc.tensor.matmul(
                    za[:], diags[k][:], xb[:, lo : lo + 512],
                    start=(i == 0), stop=(i == 5),
                )
            nc.scalar.copy(z[:, 0:512], za[:])
            nc.sync.dma_start(z_d[j][:, 0:512], z[:, 0:512])
            zb = pre_ps.tile([128, 512], f32, tag="ps", name=f"zb_{j}")
            nc.tensor.matmul(zb[:], diags[0][:], xb[:, 512:1024], start=True, stop=False)
            nc.tensor.matmul(zb[:], diags[4][:], xb[:, 3072:3584], start=False, stop=True)
            nc.scalar.copy(z[:, 512:1024], zb[:])
            nc.sync.dma_start(z_d[j][:, 512:DW], z[:, 512:DW])

        def emit_stage23(ci):
            c0, c1 = CHUNKS[ci]
            w = c1 - c0
            nj = w // 128
            jbase = c0 // 128
            tail = ci == len(CHUNKS) - 1
            sc = scs[ci]
            # ---------- softmax over the 6 nodes (batch-major) ----------
            if tail:
                # junk matmuls gated on the final scores keep the PE warm
                # through the softmax window for the tail diag-matmuls
                nc.scalar.copy(junk_x[0:1, 0:1], sc[0:1, 0:1])
                jp2 = score_ps.tile([128, 512], f32, tag="sc", name="jp2")
                for i in range(N_JUNK2):
                    nc.tensor.matmul(
                        jp2[:, :JUNK_F], junk_w[:], junk_x[:], start=True, stop=True
                    )
            sc_sb = small.tile([6, w], f32, tag="sc_sb")
            nc.scalar.copy(sc_sb[:], sc[0:6, :])
            tp = tp_ps.tile([128, nj * 6], f32)
            for j in range(nj):
                nc.tensor.transpose(
                    tp[:, j * 6 : (j + 1) * 6],
                    sc_sb[:, j * 128 : (j + 1) * 128],
                    eye_t[:],
                )
            expt = small.tile([128, nj * 6], f32, tag="expt")
            sumexp = small.tile([128, nj], f32, tag="sumexp")
            nc.scalar.activation(expt[:], tp[:], Exp)
            if tail:
                # second junk batch bridges exp -> diag-build
                nc.scalar.copy(junk_x[0:1, 0:1], expt[0:1, 0:1])
                jp3 = score_ps.tile([128, 512], f32, tag="sc", name="jp3")
                for i in range(N_JUNK3):
                    nc.tensor.matmul(
                        jp3[:, :JUNK_F], junk_w[:], junk_x[:], start=True, stop=True
                    )
            nc.vector.tensor_reduce(
                sumexp[:],
                expt[:].rearrange("p (j k) -> p j k", j=nj),
                axis=mybir.AxisListType.X,
                op=mybir.AluOpType.add,
            )
            rec = small.tile([128, nj], f32, tag="rec")
            nc.vector.reciprocal(rec[:], sumexp[:])
            beta = small.tile([128, nj * 6], f32, tag="beta")
            for j in range(nj):
                nc.vector.tensor_scalar_mul(
                    beta[:, j * 6 : (j + 1) * 6],
                    expt[:, j * 6 : (j + 1) * 6],
                    rec[:, j : j + 1],
                )
            # ---------- stage 3 ----------
            if not tail:
                for jrel in range(nj):
                    dve_group(jbase + jrel, beta, jrel)
            else:
                pe_group(jbase + 1, beta, 1)
                dve_group(jbase, beta, 0, split_dma=True)

        for n in NODE_ORDER:
            emit_stage1_node(0, n)
        for ci in range(1, len(CHUNKS)):
            emit_stage1_node(ci, NODE_ORDER[0])
            emit_stage23(ci - 1)
            for n in NODE_ORDER[1:]:
                emit_stage1_node(ci, n)
        flush_sc()
        emit_stage23(len(CHUNKS) - 1)

    nc.compile()
    return nc


def _get_nc():
    key = (MM_DTYPE, S3_DTYPE, SC_DTYPE)
    if key not in _cache:
        _cache[key] = _build(*key)
    return _cache[key]


def kernel(result_ls, result_A, result_lm, result_AT, result_ds, result_dm, W, b, h_n):
    global LAST_EXEC_TIME_NS, LAST_RESULT
    _install_ntff_hook()
    from concourse.bass_utils import run_bass_kernel_spmd

    import concourse.mybir as mybir

    nc = _get_nc()
    mm_np = mybir.dt.np(getattr(mybir.dt, MM_DTYPE))
    s3_np = mybir.dt.np(getattr(mybir.dt, S3_DTYPE))
    sc_np = mybir.dt.np(getattr(mybir.dt, SC_DTYPE))

    x = np.concatenate(
        [
            np.asarray(t, dtype=np.float32).reshape(B_TOTAL, -1)
            for t in (result_ls, result_A, result_lm, result_AT, result_ds, result_dm)
        ],
        axis=1,
    )  # [8192, 4096]
    W = np.asarray(W, dtype=np.float32)
    b = np.asarray(b, dtype=np.float32)
    h_n = np.asarray(h_n, dtype=np.float32)

    wT = np.ascontiguousarray(W[:, :DW].T).astype(mm_np)       # [1024, 512]
    wt = wT.reshape(2, 4, 128, OUT).transpose(0, 2, 1, 3)      # [2, 128, 4, 512]
    wt = np.ascontiguousarray(wt)
    bias = np.ascontiguousarray(b.reshape(4, 128).T)           # [128, 4]
    # h6[p, n*4 + oc, m] = h[oc*128 + p] if m == n else 0
    h6 = np.zeros((128, 24, 16), dtype=np.float32)
    for n in range(NODES):
        for oc in range(4):
            h6[:, n * 4 + oc, n] = h_n[oc * 128 : (oc + 1) * 128, 0]
    h6 = h6.astype(sc_np)
    eye = np.eye(6, dtype=np.float32)
    eye128 = np.eye(128, dtype=s3_np)

    in_maps = []
    for c in range(NCORES):
        xc = x[c * BLOC : (c + 1) * BLOC]                      # [1024, 4096]
        xT = np.ascontiguousarray(xc.T)                        # [4096, 1024]
        # xt[g, p, jkc, b] = xT[g*512 + jkc*128 + p, b]
        xt = np.ascontiguousarray(
            xT.reshape(NG, GK, 128, BLOC).transpose(0, 2, 1, 3)
        ).astype(mm_np)
        in_maps.append(
            {
                "xt": xt,
                "xb": np.ascontiguousarray(xc.reshape(8, 128, 4096)).astype(s3_np),
                "wt": wt.reshape(2, 128, 4 * OUT),
                "bias": bias,
                "h6": h6,
                "eye": eye,
                "eye128": eye128,
            }
        )

    res = run_bass_kernel_spmd(nc, in_maps, list(range(NCORES)))
    LAST_RESULT = res
    LAST_EXEC_TIME_NS = res.exec_time_ns

    out = np.zeros((B_TOTAL, 1, 2048), dtype=np.float32)
    for c in range(NCORES):
        zc = res.results[c]["z"]                               # [8, 128, 1024]
        out[c * BLOC : (c + 1) * BLOC, 0, :DW] = zc.reshape(BLOC, DW).astype(np.float32)
    return out
